# revision 8
# baseline (speedup 1.0000x reference)
"""Causal self-attention (B=4, S=2048, D=768, H=12) on 8 trn2 NeuronCores.

Sharding: core c -> (batch b = c//2, head-half hh = c%2). Each core handles
one batch and 6 of the 12 heads: it computes qkv for its 384 q/k/v columns,
full causal attention for its 6 heads, and a partial output projection over
its 384 rows of w_proj. Host sums the two half partials per batch + b_proj.

Device pipeline (bf16 matmul operands / f32 PSUM accumulation):
  x arrives PRE-TRANSPOSED from the host (xT [768, 2048] bf16) so no PE
  transposes / DVE staging copies are needed; QT/KT 3x[128,512]-per-chunk
  pack 2 heads per 128 partitions (q pre-scaled by 1/8); VV 16x[128,390]
  are v s-chunk tiles with a ones column per head so A@V also yields the
  softmax rowsum.  Attention runs per (q-chunk c of 512) x (head-pack t):
  both heads' S^T strips (k on partitions, q on free dim) go into one
  [128,1024] PSUM tile; one ScalarE exp per strip-pair; causal mask = 0/1
  upper-triangular multiply on the diagonal block only (gpsimd); U~^T =
  V~^T @ expS^T accumulates in one [65,2,512] PSUM tile per head pack
  (row 64 = rowsum).  After each (c,t): DVE reciprocal straight off the
  PSUM rowsum rows, gpsimd partition_broadcast replicates the reciprocal
  across partitions, and two DVE tensor_tensor ops extract+normalize U^T
  in one pass; after each c: partial projection into a [128,1024] PSUM
  tile, one DVE drain to bf16, output DMA (host accumulates in f32).
"""

import numpy as np

B, S, D, H, HD = 4, 2048, 768, 12, 64
HPC = 6  # heads per core
N_CORES = 8

_built_nc = None


def _build():
    import concourse.bass as bass
    import concourse.mybir as mybir
    from concourse import bacc
    import concourse.tile as tile
    from concourse.masks import make_upper_triangular
    from contextlib import ExitStack

    f32 = mybir.dt.float32
    bf16 = mybir.dt.bfloat16
    FT = mybir.ActivationFunctionType
    MUL = mybir.AluOpType.mult

    nc = bacc.Bacc("TRN2", target_bir_lowering=False, debug=False)
    # x arrives pre-transposed + pre-cast to bf16 from the host
    xT_d = nc.dram_tensor("xT_in", [D, S], bf16, kind="ExternalInput").ap()
    w_d = nc.dram_tensor("w_in", [D, 1152], bf16, kind="ExternalInput").ap()
    bqkv_d = nc.dram_tensor("bqkv_in", [1152], f32, kind="ExternalInput").ap()
    wp_d = nc.dram_tensor("wp_in", [384, D], bf16, kind="ExternalInput").ap()
    out_d = nc.dram_tensor("out", [S, D], bf16, kind="ExternalOutput").ap()

    with tile.TileContext(nc) as tc, ExitStack() as ctx:
        # ---------------- constants + persistent tiles ----------------
        pconst = ctx.enter_context(tc.tile_pool(name="const", bufs=1))
        utri = pconst.tile([128, 128], bf16)  # 1.0 where p <= c else 0.0
        make_upper_triangular(nc, utri[:], val=1.0, diag=True)
        bq = pconst.tile([128, 6], f32)  # per-chunk bias vecs: cols 0-2 q, 3-5 k
        nc.scalar.dma_start(bq[:], bqkv_d[0:768].rearrange("(c p) -> p c", p=128))
        bv_row = pconst.tile([1, 384], f32)
        nc.scalar.dma_start(bv_row[:], bqkv_d[768:1152].rearrange("(o n) -> o n", o=1))
        bvb = pconst.tile([128, 384], f32)  # bias_v broadcast to 128 partitions
        nc.gpsimd.partition_broadcast(bvb[:], bv_row[:])

        pqkv = ctx.enter_context(tc.tile_pool(name="qkvout", bufs=1))
        # QT/KT split per 512-col s-chunk so attention chunk c only depends on
        # the matching qkv chunk (Tile deps are tile-granular)
        QT = [
            [pqkv.tile([128, 512], bf16, name=f"qt{t}_{sc}") for sc in range(4)]
            for t in range(3)
        ]
        KT = [
            [pqkv.tile([128, 512], bf16, name=f"kt{t}_{sc}") for sc in range(4)]
            for t in range(3)
        ]
        VV = [pqkv.tile([128, HPC * 65], bf16, name=f"vv{i}") for i in range(16)]
        UT = [pqkv.tile([128, S], bf16, name=f"ut{t}") for t in range(3)]
        wpt = pqkv.tile([128, 3, D], bf16)
        pes = ctx.enter_context(tc.tile_pool(name="espool", bufs=6))
        pnrm = ctx.enter_context(tc.tile_pool(name="nrm", bufs=3))
        prr = ctx.enter_context(tc.tile_pool(name="rrp", bufs=4))
        pout = ctx.enter_context(tc.tile_pool(name="outp", bufs=4))

        # attention PSUM (6 banks) + shared 2-bank qkv/mm pool
        pst2 = ctx.enter_context(tc.tile_pool(name="stps", space="PSUM", bufs=2))
        pav = ctx.enter_context(tc.tile_pool(name="avps", space="PSUM", bufs=1))
        pmm = ctx.enter_context(tc.tile_pool(name="mmps", space="PSUM", bufs=2))

        # ------- interleaved: per 512-chunk qkv production + attention -------
        p1 = ctx.enter_context(tc.tile_pool(name="ph1", bufs=1))
        wt = p1.tile([128, 6, 1152], bf16)
        # split the weight DMA by q/k/v column group so the first Q/K matmuls
        # only wait on 0.59MB each instead of the full 1.77MB; weights go on
        # the Activation-hosted HWDGE queue, x/out on the SP queue, so the
        # startup loads run concurrently
        for g in range(3):
            nc.scalar.dma_start(
                wt[:, :, g * 384 : (g + 1) * 384],
                w_d[:, g * 384 : (g + 1) * 384].rearrange("(c p) n -> p c n", p=128),
            )
        # x^T tiles, one per 512-token chunk (DMA'd directly - no transposes)
        xt = [p1.tile([128, 6, 512], bf16, name=f"xt{sc}") for sc in range(4)]
        for sc in range(4):
            nc.sync.dma_start(
                xt[sc][:],
                xT_d[:, sc * 512 : (sc + 1) * 512].rearrange("(c p) s -> p c s", p=128),
            )
        nc.scalar.dma_start(wpt[:], wp_d.rearrange("(c p) n -> p c n", p=128))

        for sc in range(4):
            # -- qkv chunk sc: V + QT/KT --
            i0 = sc * 4
            for i in range(i0, i0 + 4):
                psv = pmm.tile([128, 384], f32, tag="mm")
                for c in range(6):
                    nc.tensor.matmul(
                        psv[:],
                        lhsT=xt[sc][:, c, (i - i0) * 128 : (i - i0 + 1) * 128],
                        rhs=wt[:, c, 768:1152],
                        start=(c == 0),
                        stop=(c == 5),
                    )
                vt = VV[i][:].rearrange("p (h m) -> p h m", m=65)
                nc.vector.tensor_tensor(
                    vt[:, :, 0:64],
                    psv[:].rearrange("p (h m) -> p h m", m=64),
                    bvb[:].rearrange("p (h m) -> p h m", m=64),
                    mybir.AluOpType.add,
                )
                nc.vector.memset(vt[:, :, 64:65], 1.0)
            for ncI in range(3):
                for which, dst in ((0, QT), (1, KT)):
                    base = which * 384
                    ps = pmm.tile([128, 512], f32, tag="mm")
                    for c in range(6):
                        nc.tensor.matmul(
                            ps[:],
                            lhsT=wt[:, c, base + ncI * 128 : base + (ncI + 1) * 128],
                            rhs=xt[sc][:, c, :],
                            start=(c == 0),
                            stop=(c == 5),
                        )
                    cidx = which * 3 + ncI
                    # drain on DVE: the ACT FIFO is saturated with exp calls,
                    # a drain queued behind them stalls the mm-slot ring
                    nc.vector.tensor_scalar_add(
                        dst[ncI][sc][:],
                        ps[:],
                        bq[:, cidx : cidx + 1],
                    )

            # -- attention / norm / projection for chunk c == sc --
            c = sc
            g0 = c * 512  # global q base of this chunk
            for t in range(3):
                av = pav.tile([65, 2, 512], f32, tag="av")
                for j in range(4 * c + 4):
                    n0 = max(0, j * 128 - g0)
                    W = 512 - n0
                    jc, jr = j // 4, (j % 4) * 128
                    # ScalarE's exp stream is the kernel bottleneck: schedule
                    # the score matmuls + exp at max priority so PE keeps the
                    # ACT queue fed; qkv/AV/proj matmuls fill PE gaps
                    with tc.high_priority():
                        st = pst2.tile([128, 1024], f32, tag="st")
                        nc.tensor.matmul(
                            st[:, 0:W],
                            lhsT=KT[t][jc][0:64, jr : jr + 128],
                            rhs=QT[t][c][0:64, n0:512],
                            start=True,
                            stop=True,
                        )
                        nc.tensor.matmul(
                            st[:, 512 : 512 + W],
                            lhsT=KT[t][jc][64:128, jr : jr + 128],
                            rhs=QT[t][c][64:128, n0:512],
                            start=True,
                            stop=True,
                        )
                        es = pes.tile([128, 1024], bf16, tag="es")
                        nc.scalar.activation(
                            es[:].rearrange("p (h w) -> p h w", h=2)[:, :, 0:W],
                            st[:].rearrange("p (h w) -> p h w", h=2)[:, :, 0:W],
                            FT.Exp,
                        )
                    if j * 128 >= g0:  # diagonal block at start of valid region
                        nc.gpsimd.tensor_tensor(
                            es[:, 0:128], es[:, 0:128], utri[:], MUL
                        )
                        nc.gpsimd.tensor_tensor(
                            es[:, 512:640], es[:, 512:640], utri[:], MUL
                        )
                    last = j == 4 * c + 3
                    nc.tensor.matmul(
                        av[:, 0, n0:512],
                        lhsT=VV[j][:, (2 * t) * 65 : (2 * t + 1) * 65],
                        rhs=es[:, 0:W],
                        start=(j == 0),
                        stop=last,
                    )
                    nc.tensor.matmul(
                        av[:, 1, n0:512],
                        lhsT=VV[j][:, (2 * t + 1) * 65 : (2 * t + 2) * 65],
                        rhs=es[:, 512 : 512 + W],
                        start=(j == 0),
                        stop=last,
                    )
                # normalize + extract U^T: copy both heads' PSUM rowsum rows
                # to partition 0 (custom-DVE ops require base-0 partitions),
                # reciprocal once, broadcast across partitions on gpsimd,
                # then one fused multiply per head straight out of PSUM
                rs = prr.tile([1, 2, 512], f32, tag="rr")
                nc.vector.tensor_copy(rs[:], av[64:65, :, :])
                rsr = prr.tile([1, 2, 512], f32, tag="rr")
                nc.vector.reciprocal_approx_fast(rsr[:], rs[:])
                recA = pnrm.tile([64, 512], f32, tag="rec")
                recB = pnrm.tile([64, 512], f32, tag="rec")
                nc.gpsimd.partition_broadcast(recA[:], rsr[0:1, 0, :])
                nc.gpsimd.partition_broadcast(recB[:], rsr[0:1, 1, :])
                nc.vector.tensor_tensor(
                    UT[t][0:64, g0 : g0 + 512], av[0:64, 0, :], recA[:], MUL
                )
                nc.vector.tensor_tensor(
                    UT[t][64:128, g0 : g0 + 512], av[0:64, 1, :], recB[:], MUL
                )
            # partial projection + store for this chunk's 4 s-tiles.
            # Alternate the PSUM tag between the strip ring and the (now
            # idle) AV slot so the four projections pipeline 2-wide instead
            # of serializing behind the last strips' slots
            for i in range(4 * c, 4 * c + 4):
                ppool, ptag = (pst2, "st") if i % 2 == 0 else (pav, "av")
                po = ppool.tile([128, 1024], f32, tag=ptag)
                for t in range(3):
                    nc.tensor.matmul(
                        po[:, 0:512],
                        lhsT=UT[t][:, i * 128 : (i + 1) * 128],
                        rhs=wpt[:, t, 0:512],
                        start=(t == 0),
                        stop=(t == 2),
                    )
                for t in range(3):
                    nc.tensor.matmul(
                        po[:, 512:768],
                        lhsT=UT[t][:, i * 128 : (i + 1) * 128],
                        rhs=wpt[:, t, 512:768],
                        start=(t == 0),
                        stop=(t == 2),
                    )
                ob = pout.tile([128, D], bf16, tag="ob")
                nc.vector.tensor_copy(ob[:], po[:, 0:768])
                nc.sync.dma_start(out_d[i * 128 : (i + 1) * 128, :], ob[:])

    nc.compile()
    return nc


def _get_nc():
    global _built_nc
    if _built_nc is None:
        _built_nc = _build()
    return _built_nc


def _make_in_maps(x, w_qkv, b_qkv, w_proj):
    import ml_dtypes

    bf16 = ml_dtypes.bfloat16
    in_maps = []
    xTb = [np.ascontiguousarray(x[b].T.astype(bf16)) for b in range(B)]
    for core in range(N_CORES):
        b, hh = core // 2, core % 2
        cs = slice(hh * 384, (hh + 1) * 384)
        wq = w_qkv[:, 0:768][:, cs] * np.float32(0.125)  # fold 1/sqrt(64)
        wk = w_qkv[:, 768:1536][:, cs]
        wv = w_qkv[:, 1536:2304][:, cs]
        w_in = np.ascontiguousarray(
            np.concatenate([wq, wk, wv], axis=1).astype(bf16)
        )
        bqv = np.concatenate(
            [
                b_qkv[0:768][cs] * np.float32(0.125),
                b_qkv[768:1536][cs],
                b_qkv[1536:2304][cs],
            ]
        ).astype(np.float32)
        wp = np.ascontiguousarray(w_proj[cs, :].astype(bf16))
        in_maps.append(
            {
                "xT_in": xTb[b],
                "w_in": w_in,
                "bqkv_in": bqv,
                "wp_in": wp,
            }
        )
    return in_maps


def _run(x, w_qkv, b_qkv, w_proj, b_proj, trace=False):
    from concourse.bass_utils import run_bass_kernel_spmd

    nc = _get_nc()
    in_maps = _make_in_maps(x, w_qkv, b_qkv, w_proj)
    res = run_bass_kernel_spmd(
        nc, in_maps, core_ids=list(range(N_CORES)), trace=trace
    )
    out = np.zeros((B, S, D), np.float32)
    for core in range(N_CORES):
        out[core // 2] += np.asarray(res.results[core]["out"], np.float32)
    out += np.asarray(b_proj, np.float32)[None, None, :]
    return out, res


def kernel(**inputs):
    x = np.asarray(inputs["x"], np.float32)
    w_qkv = np.asarray(inputs["w_qkv"], np.float32)
    b_qkv = np.asarray(inputs["b_qkv"], np.float32)
    w_proj = np.asarray(inputs["w_proj"], np.float32)
    b_proj = np.asarray(inputs["b_proj"], np.float32)
    out, _ = _run(x, w_qkv, b_qkv, w_proj, b_proj, trace=False)
    return out


# revision 11
# speedup vs baseline: 1.0570x; 1.0570x over previous
"""Causal self-attention (B=4, S=2048, D=768, H=12) on 8 trn2 NeuronCores.

Sharding: core c -> (batch b = c//2, head-half hh = c%2). Each core handles
one batch and 6 of the 12 heads: it computes qkv for its 384 q/k/v columns,
full causal attention for its 6 heads, and a partial output projection over
its 384 rows of w_proj. Host sums the two half partials per batch + b_proj.

Device pipeline (bf16 matmul operands / f32 PSUM accumulation):
  x arrives PRE-TRANSPOSED from the host (xT [768, 2048] bf16) so no PE
  transposes / DVE staging copies are needed; QT/KT 3x[128,512]-per-chunk
  pack 2 heads per 128 partitions (q pre-scaled by 1/8); VV 16x[128,390]
  are v s-chunk tiles with a ones column per head so A@V also yields the
  softmax rowsum.  Attention runs per (q-chunk c of 512) x (head-pack t):
  both heads' S^T strips (k on partitions, q on free dim) go into one
  [128,1024] PSUM tile; one ScalarE exp per strip-pair; causal mask = 0/1
  upper-triangular multiply on the diagonal block only (gpsimd); U~^T =
  V~^T @ expS^T accumulates in one [65,2,512] PSUM tile per head pack
  (row 64 = rowsum).  After each (c,t): DVE reciprocal straight off the
  PSUM rowsum rows, gpsimd partition_broadcast replicates the reciprocal
  across partitions, and two DVE tensor_tensor ops extract+normalize U^T
  in one pass; after each c: partial projection into a [128,1024] PSUM
  tile, one DVE drain to bf16, output DMA (host accumulates in f32).
"""

import numpy as np

B, S, D, H, HD = 4, 2048, 768, 12, 64
HPC = 6  # heads per core
N_CORES = 8

_built_nc = None


def _build():
    import concourse.bass as bass
    import concourse.mybir as mybir
    from concourse import bacc
    import concourse.tile as tile
    from concourse.masks import make_upper_triangular
    from contextlib import ExitStack

    f32 = mybir.dt.float32
    bf16 = mybir.dt.bfloat16
    FT = mybir.ActivationFunctionType
    MUL = mybir.AluOpType.mult

    nc = bacc.Bacc("TRN2", target_bir_lowering=False, debug=False)
    # x arrives pre-transposed + pre-cast to bf16 from the host
    xT_d = nc.dram_tensor("xT_in", [D, S], bf16, kind="ExternalInput").ap()
    w_d = nc.dram_tensor("w_in", [D, 1152], bf16, kind="ExternalInput").ap()
    bqkv_d = nc.dram_tensor("bqkv_in", [1152], f32, kind="ExternalInput").ap()
    wp_d = nc.dram_tensor("wp_in", [384, D], bf16, kind="ExternalInput").ap()
    out_d = nc.dram_tensor("out", [S, D], bf16, kind="ExternalOutput").ap()

    with tile.TileContext(nc) as tc, ExitStack() as ctx:
        # ---------------- constants + persistent tiles ----------------
        pconst = ctx.enter_context(tc.tile_pool(name="const", bufs=1))
        utri = pconst.tile([128, 128], bf16)  # 1.0 where p <= c else 0.0
        make_upper_triangular(nc, utri[:], val=1.0, diag=True)
        bq = pconst.tile([128, 6], f32)  # per-chunk bias vecs: cols 0-2 q, 3-5 k
        bv_row = pconst.tile([1, 384], f32)
        bvb = pconst.tile([128, 384], f32)  # bias_v broadcast to 128 partitions

        pqkv = ctx.enter_context(tc.tile_pool(name="qkvout", bufs=1))
        # QT/KT split per 512-col s-chunk so attention chunk c only depends on
        # the matching qkv chunk (Tile deps are tile-granular)
        QT = [
            [pqkv.tile([128, 512], bf16, name=f"qt{t}_{sc}") for sc in range(4)]
            for t in range(3)
        ]
        KT = [
            [pqkv.tile([128, 512], bf16, name=f"kt{t}_{sc}") for sc in range(4)]
            for t in range(3)
        ]
        VV = [pqkv.tile([128, HPC * 65], bf16, name=f"vv{i}") for i in range(16)]
        UT = [pqkv.tile([128, S], bf16, name=f"ut{t}") for t in range(3)]
        wpt = pqkv.tile([128, 3, D], bf16)
        pes = ctx.enter_context(tc.tile_pool(name="espool", bufs=6))
        pnrm = ctx.enter_context(tc.tile_pool(name="nrm", bufs=3))
        prr = ctx.enter_context(tc.tile_pool(name="rrp", bufs=4))
        pout = ctx.enter_context(tc.tile_pool(name="outp", bufs=4))

        # attention PSUM (6 banks) + shared 2-bank qkv/mm pool
        pst2 = ctx.enter_context(tc.tile_pool(name="stps", space="PSUM", bufs=2))
        pav = ctx.enter_context(tc.tile_pool(name="avps", space="PSUM", bufs=1))
        pmm = ctx.enter_context(tc.tile_pool(name="mmps", space="PSUM", bufs=2))

        # ------- interleaved: per 512-chunk qkv production + attention -------
        p1 = ctx.enter_context(tc.tile_pool(name="ph1", bufs=1))
        wt = p1.tile([128, 6, 1152], bf16)
        xt = [p1.tile([128, 6, 512], bf16, name=f"xt{sc}") for sc in range(4)]
        # The cost model serializes all transfers through one DMA lane, so
        # issue order ~= arrival order.  Gate-first: xt0 (SP queue) || wq, wk
        # (ACT queue), then everything else in need order.
        nc.sync.dma_start(
            xt[0][:], xT_d[:, 0:512].rearrange("(c p) s -> p c s", p=128)
        )
        for g in range(2):  # wq, wk
            nc.scalar.dma_start(
                wt[:, :, g * 384 : (g + 1) * 384],
                w_d[:, g * 384 : (g + 1) * 384].rearrange("(c p) n -> p c n", p=128),
            )
        nc.scalar.dma_start(bq[:], bqkv_d[0:768].rearrange("(c p) -> p c", p=128))
        nc.scalar.dma_start(
            bv_row[:], bqkv_d[768:1152].rearrange("(o n) -> o n", o=1)
        )
        nc.gpsimd.partition_broadcast(bvb[:], bv_row[:])
        nc.scalar.dma_start(  # wv
            wt[:, :, 768:1152], w_d[:, 768:1152].rearrange("(c p) n -> p c n", p=128)
        )
        for sc in range(1, 4):
            nc.scalar.dma_start(
                xt[sc][:],
                xT_d[:, sc * 512 : (sc + 1) * 512].rearrange("(c p) s -> p c s", p=128),
            )
        nc.scalar.dma_start(wpt[:], wp_d.rearrange("(c p) n -> p c n", p=128))

        for sc in range(4):
            # -- qkv chunk sc: QT/KT first (they gate the exp stream), V last --
            i0 = sc * 4
            for ncI in range(3):
                for which, dst in ((0, QT), (1, KT)):
                    base = which * 384
                    ps = pmm.tile([128, 512], f32, tag="mm")
                    for c in range(6):
                        nc.tensor.matmul(
                            ps[:],
                            lhsT=wt[:, c, base + ncI * 128 : base + (ncI + 1) * 128],
                            rhs=xt[sc][:, c, :],
                            start=(c == 0),
                            stop=(c == 5),
                        )
                    cidx = which * 3 + ncI
                    # drain on ScalarE at high priority: ACT's FIFO is full
                    # of exp calls, and a drain stuck behind them stalls the
                    # mm-slot ring (and with it the PE)
                    with tc.high_priority():
                        nc.scalar.activation(
                            dst[ncI][sc][:],
                            ps[:],
                            FT.Identity,
                            bias=bq[:, cidx : cidx + 1],
                        )
            for i in range(i0, i0 + 4):
                psv = pmm.tile([128, 384], f32, tag="mm")
                for c in range(6):
                    nc.tensor.matmul(
                        psv[:],
                        lhsT=xt[sc][:, c, (i - i0) * 128 : (i - i0 + 1) * 128],
                        rhs=wt[:, c, 768:1152],
                        start=(c == 0),
                        stop=(c == 5),
                    )
                vt = VV[i][:].rearrange("p (h m) -> p h m", m=65)
                nc.vector.tensor_tensor(
                    vt[:, :, 0:64],
                    psv[:].rearrange("p (h m) -> p h m", m=64),
                    bvb[:].rearrange("p (h m) -> p h m", m=64),
                    mybir.AluOpType.add,
                )
                nc.vector.memset(vt[:, :, 64:65], 1.0)

            # -- attention / norm / projection for chunk c == sc --
            c = sc
            g0 = c * 512  # global q base of this chunk
            for t in range(3):
                av = pav.tile([65, 2, 512], f32, tag="av")
                for j in range(4 * c + 4):
                    n0 = max(0, j * 128 - g0)
                    W = 512 - n0
                    jc, jr = j // 4, (j % 4) * 128
                    # ScalarE's exp stream is the kernel bottleneck: schedule
                    # the score matmuls + exp at max priority so PE keeps the
                    # ACT queue fed; qkv/AV/proj matmuls fill PE gaps
                    with tc.high_priority():
                        st = pst2.tile([128, 1024], f32, tag="st")
                        nc.tensor.matmul(
                            st[:, 0:W],
                            lhsT=KT[t][jc][0:64, jr : jr + 128],
                            rhs=QT[t][c][0:64, n0:512],
                            start=True,
                            stop=True,
                        )
                        nc.tensor.matmul(
                            st[:, 512 : 512 + W],
                            lhsT=KT[t][jc][64:128, jr : jr + 128],
                            rhs=QT[t][c][64:128, n0:512],
                            start=True,
                            stop=True,
                        )
                        es = pes.tile([128, 1024], bf16, tag="es")
                        nc.scalar.activation(
                            es[:].rearrange("p (h w) -> p h w", h=2)[:, :, 0:W],
                            st[:].rearrange("p (h w) -> p h w", h=2)[:, :, 0:W],
                            FT.Exp,
                        )
                    if j * 128 >= g0:  # diagonal block at start of valid region
                        nc.gpsimd.tensor_tensor(
                            es[:, 0:128], es[:, 0:128], utri[:], MUL
                        )
                        nc.gpsimd.tensor_tensor(
                            es[:, 512:640], es[:, 512:640], utri[:], MUL
                        )
                    last = j == 4 * c + 3
                    nc.tensor.matmul(
                        av[:, 0, n0:512],
                        lhsT=VV[j][:, (2 * t) * 65 : (2 * t + 1) * 65],
                        rhs=es[:, 0:W],
                        start=(j == 0),
                        stop=last,
                    )
                    nc.tensor.matmul(
                        av[:, 1, n0:512],
                        lhsT=VV[j][:, (2 * t + 1) * 65 : (2 * t + 2) * 65],
                        rhs=es[:, 512 : 512 + W],
                        start=(j == 0),
                        stop=last,
                    )
                # normalize + extract U^T: copy both heads' PSUM rowsum rows
                # to partition 0 (custom-DVE ops require base-0 partitions),
                # reciprocal once, broadcast across partitions on gpsimd,
                # then one fused multiply per head straight out of PSUM
                rs = prr.tile([1, 2, 512], f32, tag="rr")
                nc.vector.tensor_copy(rs[:], av[64:65, :, :])
                rsr = prr.tile([1, 2, 512], f32, tag="rr")
                nc.vector.reciprocal_approx_fast(rsr[:], rs[:])
                recA = pnrm.tile([64, 512], f32, tag="rec")
                recB = pnrm.tile([64, 512], f32, tag="rec")
                nc.gpsimd.partition_broadcast(recA[:], rsr[0:1, 0, :])
                nc.gpsimd.partition_broadcast(recB[:], rsr[0:1, 1, :])
                nc.vector.tensor_tensor(
                    UT[t][0:64, g0 : g0 + 512], av[0:64, 0, :], recA[:], MUL
                )
                nc.vector.tensor_tensor(
                    UT[t][64:128, g0 : g0 + 512], av[0:64, 1, :], recB[:], MUL
                )
            # partial projection + store for this chunk's 4 s-tiles.
            # Alternate the PSUM tag between the strip ring and the (now
            # idle) AV slot so the four projections pipeline 2-wide instead
            # of serializing behind the last strips' slots
            for i in range(4 * c, 4 * c + 4):
                ppool, ptag = (pst2, "st") if i % 2 == 0 else (pav, "av")
                po = ppool.tile([128, 1024], f32, tag=ptag)
                for t in range(3):
                    nc.tensor.matmul(
                        po[:, 0:512],
                        lhsT=UT[t][:, i * 128 : (i + 1) * 128],
                        rhs=wpt[:, t, 0:512],
                        start=(t == 0),
                        stop=(t == 2),
                    )
                for t in range(3):
                    nc.tensor.matmul(
                        po[:, 512:768],
                        lhsT=UT[t][:, i * 128 : (i + 1) * 128],
                        rhs=wpt[:, t, 512:768],
                        start=(t == 0),
                        stop=(t == 2),
                    )
                ob = pout.tile([128, D], bf16, tag="ob")
                nc.vector.tensor_copy(ob[:], po[:, 0:768])
                nc.sync.dma_start(out_d[i * 128 : (i + 1) * 128, :], ob[:])

    nc.compile()
    return nc


def _get_nc():
    global _built_nc
    if _built_nc is None:
        _built_nc = _build()
    return _built_nc


def _make_in_maps(x, w_qkv, b_qkv, w_proj):
    import ml_dtypes

    bf16 = ml_dtypes.bfloat16
    in_maps = []
    xTb = [np.ascontiguousarray(x[b].T.astype(bf16)) for b in range(B)]
    for core in range(N_CORES):
        b, hh = core // 2, core % 2
        cs = slice(hh * 384, (hh + 1) * 384)
        wq = w_qkv[:, 0:768][:, cs] * np.float32(0.125)  # fold 1/sqrt(64)
        wk = w_qkv[:, 768:1536][:, cs]
        wv = w_qkv[:, 1536:2304][:, cs]
        w_in = np.ascontiguousarray(
            np.concatenate([wq, wk, wv], axis=1).astype(bf16)
        )
        bqv = np.concatenate(
            [
                b_qkv[0:768][cs] * np.float32(0.125),
                b_qkv[768:1536][cs],
                b_qkv[1536:2304][cs],
            ]
        ).astype(np.float32)
        wp = np.ascontiguousarray(w_proj[cs, :].astype(bf16))
        in_maps.append(
            {
                "xT_in": xTb[b],
                "w_in": w_in,
                "bqkv_in": bqv,
                "wp_in": wp,
            }
        )
    return in_maps


def _run(x, w_qkv, b_qkv, w_proj, b_proj, trace=False):
    from concourse.bass_utils import run_bass_kernel_spmd

    nc = _get_nc()
    in_maps = _make_in_maps(x, w_qkv, b_qkv, w_proj)
    res = run_bass_kernel_spmd(
        nc, in_maps, core_ids=list(range(N_CORES)), trace=trace
    )
    out = np.zeros((B, S, D), np.float32)
    for core in range(N_CORES):
        out[core // 2] += np.asarray(res.results[core]["out"], np.float32)
    out += np.asarray(b_proj, np.float32)[None, None, :]
    return out, res


def kernel(**inputs):
    x = np.asarray(inputs["x"], np.float32)
    w_qkv = np.asarray(inputs["w_qkv"], np.float32)
    b_qkv = np.asarray(inputs["b_qkv"], np.float32)
    w_proj = np.asarray(inputs["w_proj"], np.float32)
    b_proj = np.asarray(inputs["b_proj"], np.float32)
    out, _ = _run(x, w_qkv, b_qkv, w_proj, b_proj, trace=False)
    return out
